# revision 30
# baseline (speedup 1.0000x reference)
"""Multi-head attention (B=2, T=2048, D=1024, H=16, Dh=64) on 8 TRN2 NeuronCores.

Sharding: core c = 4*b + g  ->  batch b in {0,1}, head-group g in {0..3}
(4 heads per core: data parallel on batch, tensor parallel on heads).
Each core computes, for its batch element and its 4 heads:

  Q.T/K.T = Wq/k_shard.T @ x.T + b      [256, 2048]  (head-dim on partitions)
  V'      = x @ Wv_interleaved + b      [2048, 260]  ([V_h | 1] per head)
  per head pair (2p, 2p+1), per 512-wide i-chunk:
    S.T   = K_h Q_h.T                   (two K=64 matmuls on disjoint PE
                                         row groups -> run concurrently)
    P.T   = exp(S.T / 8)                (no max-subtraction: |S|/8 <~ 6)
    acc   = [V_h | 1].T @ P.T           [65, 512]  row 64 = softmax denom
    attnT = acc[:64] * (1/acc[64])
  partial = attnT.T @ Wout_shard        [2048, 1024]

The partial sum over the 4 head groups plus b_out is done on the host
("all-reduce after out_proj"), as is the batch unshard.

Matmuls run in fp16 (1 pass/row on the PE; fp32 PSUM accumulate).
The kernel is organized as one flat software pipeline over the 8
attention groups: the scalar engine's exp stream never breaks; V/Q/K
projection chunks, out-projection tiles, and softmax normalizations are
emitted as filler work inside the attention j-loops.
"""

import os
import numpy as np

B, T, D = 2, 2048, 1024
H, DH = 16, 64
NCORES, GROUPS = 8, 4
HPC = H // GROUPS        # 4 heads per core
F = HPC * DH             # 256 features per core
FT = F // 128            # 2 feature tiles / head pairs
KTN = D // 128           # 8 contraction tiles
TT = T // 128            # 16 token tiles
NCH = 512                # matmul free-dim chunk
VW = DH + 1              # 65: V plus ones column
VF = HPC * VW            # 260: interleaved [V_h | 1] x 4 heads

_prog = None
LAST_RESULT = None


def _build():
    from contextlib import ExitStack

    import concourse.mybir as mybir
    import concourse.tile as tile
    from concourse import bacc
    from concourse.bass import ts, AP

    f32 = mybir.dt.float32
    f32r = mybir.dt.float32r
    f16 = mybir.dt.float16
    Exp = mybir.ActivationFunctionType.Exp

    nc = bacc.Bacc()
    # All inputs arrive host-packed in SBUF layout (partition-major), so
    # each load is ONE dma_start: the Sync engine's ~600ns per-dispatch
    # cost, not bandwidth, dominates the pipeline fill.
    TC = T // NCH  # 4 column chunks of x
    xTp = nc.dram_tensor("xTp", [128, TC * KTN * NCH], f16, kind="ExternalInput")
    wq = nc.dram_tensor("wq", [128, KTN * F], f16, kind="ExternalInput")
    wk = nc.dram_tensor("wk", [128, KTN * F], f16, kind="ExternalInput")
    # wv/bv come pre-interleaved from the host: column h*65+64 is a zero
    # weight column whose bias is 1.0, producing the [V_h | 1] layout that
    # supplies the softmax-denominator row of the PV matmul for free.
    wv = nc.dram_tensor("wv", [128, KTN * VF], f16, kind="ExternalInput")
    mb = nc.dram_tensor("mb", [128, 2 * FT], f32, kind="ExternalInput")
    bv = nc.dram_tensor("bv", [1, VF], f16, kind="ExternalInput")
    wo = nc.dram_tensor("wo", [128, FT * D], f16, kind="ExternalInput")
    out = nc.dram_tensor("out", [T, D], f16, kind="ExternalOutput")

    with ExitStack() as ctx:
        tc = ctx.enter_context(tile.TileContext(nc))
        pers = ctx.enter_context(tc.tile_pool(name="pers", bufs=1))
        ptp = ctx.enter_context(tc.tile_pool(name="ptp", bufs=2))
        osb = ctx.enter_context(tc.tile_pool(name="osb", bufs=2))
        msc = ctx.enter_context(tc.tile_pool(name="msc", bufs=2))
        psq = ctx.enter_context(tc.tile_pool(name="psq", bufs=2, space="PSUM"))
        pss = ctx.enter_context(tc.tile_pool(name="pss", bufs=2, space="PSUM"))
        pso = ctx.enter_context(tc.tile_pool(name="pso", bufs=1, space="PSUM"))

        xt = pers.tile([128, TC, KTN, NCH], f16, tag="xt")
        wqs = pers.tile([128, KTN, F], f16, tag="wqs")
        wks = pers.tile([128, KTN, F], f16, tag="wks")
        wvs = pers.tile([128, KTN, VF], f16, tag="wvs")
        mbs = pers.tile([128, 2 * FT], f32, tag="mbs")  # [bq ft0, bq ft1, bk ft0, bk ft1]
        bvr = pers.tile([1, VF], f16, tag="bvr")
        ones_f = pers.tile([1, 128], f32, tag="ones_f")
        ones16 = pers.tile([1, 128], f16, tag="ones16")
        wos = pers.tile([128, FT, D], f16, tag="wos")
        qt = pers.tile([128, FT, T], f16, tag="qt")
        kt = pers.tile([128, FT, T], f16, tag="kt")
        vs = pers.tile([128, TT, VF], f16, tag="vs")
        at = pers.tile([128, FT, T], f16, tag="at")

        # ISA memset can't target f16; memset f32 then copy-convert
        nc.vector.memset(ones_f[:], 1.0)
        nc.vector.tensor_copy(ones16[:], ones_f[:])

        # DMA dispatch order follows first-use: wk + first x column-chunk
        # unblock the prologue K-projection; the rest streams in behind the
        # pipeline. One dispatch per logical tensor (chunk).
        nc.sync.dma_start(wks[:], wk[:])
        # first x chunk in two halves: the prologue K-projection's first
        # matmuls start as soon as k-tiles 0-3 land
        nc.sync.dma_start(xt[:, 0, 0:KTN // 2], xTp[:, 0:(KTN // 2) * NCH])
        nc.sync.dma_start(
            xt[:, 0, KTN // 2:], xTp[:, (KTN // 2) * NCH:KTN * NCH]
        )
        nc.sync.dma_start(wqs[:], wq[:])
        nc.sync.dma_start(mbs[:], mb[:])
        nc.sync.dma_start(bvr[:], bv[:])
        nc.sync.dma_start(wvs[:], wv[:])
        for c in range(1, TC):
            nc.sync.dma_start(xt[:, c], xTp[:, ts(c, KTN * NCH)])
        nc.sync.dma_start(wos[:], wo[:])

        # ---- deferred work units (emitted inside attention j-loops) ----
        def qk_chunk(wsb, bix, dst, ft, c):
            """Returns the two 4-matmul halves of one projection chunk, so a
            chunk spreads over two adjacent pipeline steps and never delays a
            step's scores by more than ~0.9us of PE time."""
            state = {}

            def half0():
                state["ps"] = psq.tile([128, NCH], f32, tag="psq", name="ps")
                for k in range(KTN // 2):
                    nc.tensor.matmul(
                        state["ps"][:],
                        wsb[:, k, ts(ft, 128)],
                        xt[:, c, k, :],
                        start=(k == 0), stop=False,
                    )

            def half1():
                ps = state["ps"]
                for k in range(KTN // 2, KTN):
                    nc.tensor.matmul(
                        ps[:],
                        wsb[:, k, ts(ft, 128)],
                        xt[:, c, k, :],
                        start=False, stop=(k == KTN - 1),
                    )
                nc.vector.tensor_scalar_add(
                    dst[:, ft, ts(c, NCH)], ps[:], mbs[:, bix + ft: bix + ft + 1]
                )
            return half0, half1

        def v_tile(t):
            def go():
                pv = psq.tile([128, VF], f32, tag="psq", name="pv")
                for k in range(KTN):
                    nc.tensor.matmul(
                        pv[:], xt[:, t // 4, k, ts(t % 4, 128)], wvs[:, k, :],
                        start=(k == 0), stop=False,
                    )
                # bias via ones-row (also writes the denominator 1.0 cols)
                nc.tensor.matmul(
                    pv[:], ones16[:, 0:128], bvr[:], start=False, stop=True
                )
                nc.vector.tensor_copy(vs[:, t, :], pv[:])
            return go

        def outproj_tile(t):
            def go():
                # partials leave as f16: halves the 8MB output drain DMA
                ob = osb.tile([128, D], f16, tag="ob", name="ob")
                for c in range(D // NCH):
                    pp = psq.tile([128, NCH], f32, tag="psq", name="pp")
                    for ft in range(FT):
                        nc.tensor.matmul(
                            pp[:],
                            at[:, ft, ts(t, 128)],
                            wos[:, ft, ts(c, NCH)],
                            start=(ft == 0), stop=(ft == FT - 1),
                        )
                    nc.vector.tensor_copy(ob[:, ts(c, NCH)], pp[:])
                nc.sync.dma_start(out[ts(t, 128), :], ob[:])
            return go

        obpre = pers.tile([128, 4, D], f32, tag="obpre")  # ft=0 partials, t=12..15

        def outproj_pre(t):
            def go():
                for c in range(D // NCH):
                    pp = psq.tile([128, NCH], f32, tag="psq", name="pq")
                    nc.tensor.matmul(
                        pp[:], at[:, 0, ts(t, 128)], wos[:, 0, ts(c, NCH)],
                        start=True, stop=True,
                    )
                    nc.vector.tensor_copy(obpre[:, t - 12, ts(c, NCH)], pp[:])
            return go

        def outproj_fin(t):
            def go():
                ob = osb.tile([128, D], f16, tag="ob", name="ob")
                for c in range(D // NCH):
                    pp = psq.tile([128, NCH], f32, tag="psq", name="pf")
                    nc.tensor.matmul(
                        pp[:], at[:, 1, ts(t, 128)], wos[:, 1, ts(c, NCH)],
                        start=True, stop=True,
                    )
                    nc.vector.tensor_add(
                        ob[:, ts(c, NCH)], pp[:], obpre[:, t - 12, ts(c, NCH)]
                    )
                nc.sync.dma_start(out[ts(t, 128), :], ob[:])
            return go

        def make_norm(p, ic, accs, dn_dve=False):
            """Softmax normalization for group (p, ic): attnT = num/denom.
            Reads the SBUF-staged copy of the PV accumulators (PSUM direct on
            the final group), so the chain runs off the critical path;
            emitted a few iterations into the NEXT group. (NB: Pool-engine
            offload was tried and reverted -- partition_broadcast lives in a
            separate GpSimd ucode library and each use costs a ~6us
            UNLOAD_LIB/LOAD_LIB pair.)"""
            def go():
                for hh in range(2):
                    sb = accs[hh]
                    # custom-DVE ops drop the partition base offset; stage
                    # the denominator row to SBUF partition 0 (DMA shifts
                    # partitions; keeps DVE readers off the staged tile --
                    # except on the final group, where latency wins)
                    dn = msc.tile([1, NCH], f32, tag="dn", bufs=2)
                    if dn_dve:
                        nc.vector.tensor_copy(dn[:], sb[DH: DH + 1, :])
                    else:
                        nc.sync.dma_start(dn[:], sb[DH: DH + 1, :])
                    rc = msc.tile([1, NCH], f32, tag="rc", bufs=2)
                    nc.vector.reciprocal_approx_fast(rc[:], dn[:])
                    # broadcast 1/denom across 64 partitions with a single
                    # DMA whose src repeats via a stride-0 free dim (keeps
                    # the broadcast off the PE and DVE entirely)
                    bsb = msc.tile([64, NCH], f32, tag="bsb")
                    rap = rc[:]
                    nc.sync.dma_start(
                        bsb[:],
                        AP(rap.tensor, rap.offset,
                           [rap.ap[0], [0, 64], [1, NCH]]),
                    )
                    dst_sl = ts(ic, NCH)
                    if hh == 0:
                        nc.vector.tensor_mul(
                            at[0:DH, p, dst_sl], sb[0:DH, :], bsb[:]
                        )
                    else:
                        # DVE lanes can't shift partitions; bounce via DMA
                        tmp = msc.tile([DH, NCH], f16, tag="tmp", bufs=2)
                        nc.vector.tensor_mul(tmp[:], sb[0:DH, :], bsb[:])
                        nc.sync.dma_start(at[64:128, p, dst_sl], tmp[:])
            return go

        def make_scores(p, ic):
            def scores(j):
                # disjoint PE row groups (partitions 0-63 / 64-127): the two
                # K=64 matmuls execute concurrently
                sc = pss.tile([128, 2 * NCH], f32, tag="sc", name="sc")
                for hh in range(2):
                    nc.tensor.matmul(
                        sc[:, ts(hh, NCH)],
                        kt[hh * 64: hh * 64 + DH, p, ts(j, 128)],
                        qt[hh * 64: hh * 64 + DH, p, ts(ic, NCH)],
                        start=True, stop=True,
                    )
                return sc
            return scores

        seq = [(p, ic) for p in range(FT) for ic in range(T // NCH)]
        scores_of = {g: make_scores(*g) for g in seq}

        # filler schedule: extras[si] = list of thunks, si = flat step index
        extras = {}
        def add(si, th):
            extras.setdefault(si, []).append(th)

        def add2(si, halves):
            add(si, halves[0])
            add(si + 1, halves[1])

        for j in range(TT):                         # g0: V proj just-in-time
            add(j, v_tile(j))   # fillers precede PV(j) (deferred), in time
        add2(1, qk_chunk(wks, FT, kt, 0, 1))
        add2(5, qk_chunk(wks, FT, kt, 0, 2))
        add2(9, qk_chunk(wks, FT, kt, 0, 3))
        add2(13, qk_chunk(wqs, 0, qt, 0, 1))
        add2(16, qk_chunk(wks, FT, kt, 1, 0))
        add2(18, qk_chunk(wqs, 0, qt, 0, 2))
        add2(20, qk_chunk(wks, FT, kt, 1, 1))
        add2(24, qk_chunk(wks, FT, kt, 1, 2))
        add2(28, qk_chunk(wks, FT, kt, 1, 3))
        add2(34, qk_chunk(wqs, 0, qt, 0, 3))
        add2(38, qk_chunk(wqs, 0, qt, 1, 0))
        add2(42, qk_chunk(wqs, 0, qt, 1, 1))
        add2(52, qk_chunk(wqs, 0, qt, 1, 2))
        add2(56, qk_chunk(wqs, 0, qt, 1, 3))
        for i in range(4):                          # out-proj, one ic behind
            add(80 + 4 + 3 * i, outproj_tile(i))
            add(96 + 4 + 3 * i, outproj_tile(4 + i))
            add(112 + 4 + 3 * i, outproj_tile(8 + i))
        # tiles 12-15 gate the epilogue: their ft=0 halves run early (at[:,0]
        # is final after norm(0,3) in group 4); only the ft=1 matmul + add
        # remains after the last norm
        add(85, outproj_pre(12))
        add(91, outproj_pre(13))
        add(101, outproj_pre(14))
        add(107, outproj_pre(15))

        # ---- prologue: just enough projection for the first group ----
        for th in qk_chunk(wks, FT, kt, 0, 0):
            th()
        for th in qk_chunk(wqs, 0, qt, 0, 0):
            th()

        def stage_accs(accs):
            """Copy finished PV accumulators PSUM->SBUF. The next group's
            first PV matmul (same PSUM banks, bufs=1) then only waits for
            these two cheap DVE copies instead of the full norm chain."""
            sb0 = msc.tile([VW, NCH], f32, tag="sb0", bufs=2)
            sb1 = msc.tile([VW, NCH], f32, tag="sb1", bufs=2)
            nc.vector.tensor_copy(sb0[:], accs[0][:])
            nc.vector.tensor_copy(sb1[:], accs[1][:])
            return (sb0, sb1)

        # ---- flat attention pipeline over all 8 groups ----
        # Step i emits: scores(i+1), act(i+1), fillers(i), PV(i-1). Each ACT
        # conservatively waits on ALL earlier-emitted matmuls, so act(i+1)
        # goes in before the step's fillers and the one-step-deferred PV pair
        # -- the scalar exp stream only ever waits on its own scores, and a
        # group-boundary PV (which waits on the acc staging copies) has two
        # act-periods of slack before it may block the PE queue.
        steps = [(gi, p, ic, j) for gi, (p, ic) in enumerate(seq)
                 for j in range(TT)]
        group_accs = {}

        def emit_act(i):
            g2, p2, ic2, j2 = steps[i]
            sc = scores_of[(p2, ic2)](j2)
            pe = ptp.tile([128, 2 * NCH], f16, tag="pe", name="pe", bufs=3)
            nc.scalar.activation(pe[:], sc[:], Exp, scale=0.125)
            return pe

        def emit_pv(i):
            gi, p, ic, j = steps[i]
            accs = group_accs[gi]
            pe = pes.pop(i)
            for hh in range(2):
                nc.tensor.matmul(
                    accs[hh][:, :],
                    vs[:, j, (2 * p + hh) * VW: (2 * p + hh + 1) * VW],
                    pe[:, ts(hh, NCH)],
                    start=(j == 0), stop=(j == TT - 1),
                )
            if j == TT - 1:
                if gi + 1 < len(seq):
                    # stage + hand off to the next group for normalization
                    add(16 * (gi + 1) + 2, make_norm(p, ic, stage_accs(accs)))
                else:
                    # last group: no successor reuses the PSUM banks, so
                    # normalize straight from PSUM (shortest chain)
                    make_norm(p, ic, accs, dn_dve=True)()

        pes = {0: emit_act(0)}
        for i, (gi, p, ic, j) in enumerate(steps):
            if i + 1 < len(steps):
                pes[i + 1] = emit_act(i + 1)
            if j == 0:
                acc0 = pso.tile([VW, NCH], f32, tag="acc0", name="acc0")
                acc1 = pso.tile([VW, NCH], f32, tag="acc1", name="acc1")
                group_accs[gi] = (acc0, acc1)
            for th in extras.get(i, ()):
                th()
            if i > 0:
                emit_pv(i - 1)
        emit_pv(len(steps) - 1)
        for t in range(12, 16):
            outproj_fin(t)()

    nc.finalize()  # Bacc.compile(): wait legalization, reg alloc, act tables
    return nc


def _get_program():
    global _prog
    if _prog is None:
        _prog = _build()
    return _prog


def kernel(x, W_qkv, b_qkv, W_out, b_out):
    global LAST_RESULT
    from concourse.bass_utils import run_bass_kernel_spmd

    x = np.asarray(x, np.float32)
    W_qkv = np.asarray(W_qkv, np.float32)
    b_qkv = np.asarray(b_qkv, np.float32)
    W_out = np.asarray(W_out, np.float32)
    b_out = np.asarray(b_out, np.float32)

    nc = _get_program()

    def pack_k(a):
        # [D, cols] -> SBUF layout [128, KTN*cols]: row p = concat_k a[k*128+p]
        cols = a.shape[1]
        return np.ascontiguousarray(
            a.reshape(KTN, 128, cols).transpose(1, 0, 2).reshape(128, KTN * cols)
        )

    # x packed per batch as [128, (c, k, NCH)] so each 512-col chunk of all
    # eight k-tiles is one contiguous DMA
    xp = {}
    for b in range(B):
        xT = x[b].T.astype(np.float16)              # [D, T]
        v = xT.reshape(KTN, 128, T // NCH, NCH).transpose(1, 2, 0, 3)
        xp[b] = np.ascontiguousarray(v.reshape(128, -1))

    in_maps = []
    for c in range(NCORES):
        b, g = divmod(c, GROUPS)
        sl = slice(g * F, (g + 1) * F)
        # interleave Wv/bv with [zero-weight, bias=1] columns at h*65+64
        wv_g = W_qkv[:, 2 * D:3 * D][:, sl]
        bv_g = b_qkv[2 * D:3 * D][sl]
        wv_i = np.zeros((D, VF), np.float16)
        bv_i = np.zeros((1, VF), np.float16)
        for h in range(HPC):
            wv_i[:, h * VW: h * VW + DH] = wv_g[:, h * DH:(h + 1) * DH]
            bv_i[0, h * VW: h * VW + DH] = bv_g[h * DH:(h + 1) * DH]
            bv_i[0, h * VW + DH] = 1.0
        mb = np.concatenate([
            b_qkv[0 * D:1 * D][sl].reshape(FT, 128).T,
            b_qkv[1 * D:2 * D][sl].reshape(FT, 128).T,
        ], axis=1)                                   # [128, 2*FT] f32
        in_maps.append({
            "xTp": xp[b],
            "wq": pack_k(W_qkv[:, 0 * D:1 * D][:, sl].astype(np.float16)),
            "wk": pack_k(W_qkv[:, 1 * D:2 * D][:, sl].astype(np.float16)),
            "wv": pack_k(wv_i),
            "mb": np.ascontiguousarray(mb),
            "bv": bv_i,
            "wo": np.ascontiguousarray(
                W_out[sl, :].astype(np.float16).reshape(FT, 128, D)
                .transpose(1, 0, 2).reshape(128, FT * D)
            ),
        })

    kw = {}
    if os.environ.get("KERNEL_TRACE") == "1":
        kw["trace"] = True
    res = run_bass_kernel_spmd(nc, in_maps, core_ids=list(range(NCORES)), **kw)
    LAST_RESULT = res

    out = np.empty((B, T, D), np.float32)
    for b in range(B):
        acc = res.results[GROUPS * b]["out"].astype(np.float32)
        for g in range(1, GROUPS):
            acc = acc + res.results[GROUPS * b + g]["out"]
        out[b] = acc + b_out
    return out



# revision 31
# speedup vs baseline: 1.1486x; 1.1486x over previous
"""Multi-head attention (B=2, T=2048, D=1024, H=16, Dh=64) on 8 TRN2 NeuronCores.

Sharding: core c = 4*b + g  ->  batch b in {0,1}, head-group g in {0..3}
(4 heads per core: data parallel on batch, tensor parallel on heads).
Each core computes, for its batch element and its 4 heads:

  Q.T/K.T = Wq/k_shard.T @ x.T + b      [256, 2048]  (head-dim on partitions)
  V'      = x @ Wv_interleaved + b      [2048, 260]  ([V_h | 1] per head)
  per head pair (2p, 2p+1), per 512-wide i-chunk:
    S.T   = K_h Q_h.T                   (two K=64 matmuls on disjoint PE
                                         row groups -> run concurrently)
    P.T   = exp(S.T / 8)                (no max-subtraction: |S|/8 <~ 6)
    acc   = [V_h | 1].T @ P.T           [65, 512]  row 64 = softmax denom
    attnT = acc[:64] * (1/acc[64])
  partial = attnT.T @ Wout_shard        [2048, 1024]

The partial sum over the 4 head groups plus b_out is done on the host
("all-reduce after out_proj"), as is the batch unshard.

Matmuls run in fp16 (1 pass/row on the PE; fp32 PSUM accumulate).
The kernel is organized as one flat software pipeline over the 8
attention groups: the scalar engine's exp stream never breaks; V/Q/K
projection chunks, out-projection tiles, and softmax normalizations are
emitted as filler work inside the attention j-loops.
"""

import os
import numpy as np

B, T, D = 2, 2048, 1024
H, DH = 16, 64
NCORES, GROUPS = 8, 4
HPC = H // GROUPS        # 4 heads per core
F = HPC * DH             # 256 features per core
FT = F // 128            # 2 feature tiles / head pairs
KTN = D // 128           # 8 contraction tiles
TT = T // 128            # 16 token tiles
NCH = 512                # matmul free-dim chunk
VW = DH + 1              # 65: V plus ones column
VF = HPC * VW            # 260: interleaved [V_h | 1] x 4 heads

_prog = None
LAST_RESULT = None


def _build():
    from contextlib import ExitStack

    import concourse.mybir as mybir
    import concourse.tile as tile
    from concourse import bacc
    from concourse.bass import ts, AP

    f32 = mybir.dt.float32
    f32r = mybir.dt.float32r
    f16 = mybir.dt.float16
    Exp = mybir.ActivationFunctionType.Exp

    nc = bacc.Bacc()
    # All inputs arrive host-packed in SBUF layout (partition-major), so
    # each load is ONE dma_start: the Sync engine's ~600ns per-dispatch
    # cost, not bandwidth, dominates the pipeline fill.
    TC = T // NCH  # 4 column chunks of x
    xTp = nc.dram_tensor("xTp", [128, TC * KTN * NCH], f16, kind="ExternalInput")
    wq = nc.dram_tensor("wq", [128, KTN * F], f16, kind="ExternalInput")
    wk = nc.dram_tensor("wk", [128, KTN * F], f16, kind="ExternalInput")
    # wv/bv come pre-interleaved from the host: column h*65+64 is a zero
    # weight column whose bias is 1.0, producing the [V_h | 1] layout that
    # supplies the softmax-denominator row of the PV matmul for free.
    wv = nc.dram_tensor("wv", [128, KTN * VF], f16, kind="ExternalInput")
    mb = nc.dram_tensor("mb", [128, 2 * FT], f32, kind="ExternalInput")
    bv = nc.dram_tensor("bv", [1, VF], f16, kind="ExternalInput")
    wo = nc.dram_tensor("wo", [128, FT * D], f16, kind="ExternalInput")
    out = nc.dram_tensor("out", [T, D], f16, kind="ExternalOutput")

    with ExitStack() as ctx:
        tc = ctx.enter_context(tile.TileContext(nc))
        pers = ctx.enter_context(tc.tile_pool(name="pers", bufs=1))
        ptp = ctx.enter_context(tc.tile_pool(name="ptp", bufs=2))
        osb = ctx.enter_context(tc.tile_pool(name="osb", bufs=2))
        msc = ctx.enter_context(tc.tile_pool(name="msc", bufs=2))
        psq = ctx.enter_context(tc.tile_pool(name="psq", bufs=2, space="PSUM"))
        pss = ctx.enter_context(tc.tile_pool(name="pss", bufs=2, space="PSUM"))
        pso = ctx.enter_context(tc.tile_pool(name="pso", bufs=1, space="PSUM"))

        xt = pers.tile([128, TC, KTN, NCH], f16, tag="xt")
        wqs = pers.tile([128, KTN, F], f16, tag="wqs")
        wks = pers.tile([128, KTN, F], f16, tag="wks")
        wvs = pers.tile([128, KTN, VF], f16, tag="wvs")
        mbs = pers.tile([128, 2 * FT], f32, tag="mbs")  # [bq ft0, bq ft1, bk ft0, bk ft1]
        bvr = pers.tile([1, VF], f16, tag="bvr")
        ones_f = pers.tile([1, 128], f32, tag="ones_f")
        ones16 = pers.tile([1, 128], f16, tag="ones16")
        wos = pers.tile([128, FT, D], f16, tag="wos")
        qt = pers.tile([128, FT, T], f16, tag="qt")
        kt = pers.tile([128, FT, T], f16, tag="kt")
        vs = pers.tile([128, TT, VF], f16, tag="vs")
        at = pers.tile([128, FT, T], f16, tag="at")

        # ISA memset can't target f16; memset f32 then copy-convert
        nc.vector.memset(ones_f[:], 1.0)
        nc.vector.tensor_copy(ones16[:], ones_f[:])

        # DMA dispatch order follows first-use: wk + first x column-chunk
        # unblock the prologue K-projection; the rest streams in behind the
        # pipeline. One dispatch per logical tensor (chunk).
        nc.sync.dma_start(wks[:], wk[:])
        # first x chunk in two halves: the prologue K-projection's first
        # matmuls start as soon as k-tiles 0-3 land
        nc.sync.dma_start(xt[:, 0, 0:KTN // 2], xTp[:, 0:(KTN // 2) * NCH])
        nc.sync.dma_start(
            xt[:, 0, KTN // 2:], xTp[:, (KTN // 2) * NCH:KTN * NCH]
        )
        nc.sync.dma_start(wqs[:], wq[:])
        nc.sync.dma_start(mbs[:], mb[:])
        nc.sync.dma_start(bvr[:], bv[:])
        nc.sync.dma_start(wvs[:], wv[:])
        for c in range(1, TC):
            nc.sync.dma_start(xt[:, c], xTp[:, ts(c, KTN * NCH)])
        nc.sync.dma_start(wos[:], wo[:])

        # ---- deferred work units (emitted inside attention j-loops) ----
        def qk_chunk(wsb, bix, dst, ft, c):
            """Returns the two 4-matmul halves of one projection chunk, so a
            chunk spreads over two adjacent pipeline steps and never delays a
            step's scores by more than ~0.9us of PE time."""
            state = {}

            def half0():
                state["ps"] = psq.tile([128, NCH], f32, tag="psq", name="ps")
                for k in range(KTN // 2):
                    nc.tensor.matmul(
                        state["ps"][:],
                        wsb[:, k, ts(ft, 128)],
                        xt[:, c, k, :],
                        start=(k == 0), stop=False,
                    )

            def half1():
                ps = state["ps"]
                for k in range(KTN // 2, KTN):
                    nc.tensor.matmul(
                        ps[:],
                        wsb[:, k, ts(ft, 128)],
                        xt[:, c, k, :],
                        start=False, stop=(k == KTN - 1),
                    )
                nc.vector.tensor_scalar_add(
                    dst[:, ft, ts(c, NCH)], ps[:], mbs[:, bix + ft: bix + ft + 1]
                )
            return half0, half1

        def v_tile(t):
            def go():
                pv = psq.tile([128, VF], f32, tag="psq", name="pv")
                for k in range(KTN):
                    nc.tensor.matmul(
                        pv[:], xt[:, t // 4, k, ts(t % 4, 128)], wvs[:, k, :],
                        start=(k == 0), stop=False,
                    )
                # bias via ones-row (also writes the denominator 1.0 cols)
                nc.tensor.matmul(
                    pv[:], ones16[:, 0:128], bvr[:], start=False, stop=True
                )
                nc.vector.tensor_copy(vs[:, t, :], pv[:])
            return go

        def outproj_tile(t):
            def go():
                # partials leave as f16: halves the 8MB output drain DMA
                ob = osb.tile([128, D], f16, tag="ob", name="ob")
                for c in range(D // NCH):
                    pp = psq.tile([128, NCH], f32, tag="psq", name="pp")
                    for ft in range(FT):
                        nc.tensor.matmul(
                            pp[:],
                            at[:, ft, ts(t, 128)],
                            wos[:, ft, ts(c, NCH)],
                            start=(ft == 0), stop=(ft == FT - 1),
                        )
                    nc.vector.tensor_copy(ob[:, ts(c, NCH)], pp[:])
                nc.sync.dma_start(out[ts(t, 128), :], ob[:])
            return go

        obpre = pers.tile([128, 4, D], f32, tag="obpre")  # ft=0 partials, t=12..15

        def outproj_pre(t):
            def go():
                for c in range(D // NCH):
                    pp = psq.tile([128, NCH], f32, tag="psq", name="pq")
                    nc.tensor.matmul(
                        pp[:], at[:, 0, ts(t, 128)], wos[:, 0, ts(c, NCH)],
                        start=True, stop=True,
                    )
                    nc.vector.tensor_copy(obpre[:, t - 12, ts(c, NCH)], pp[:])
            return go

        def outproj_fin(t):
            def go():
                ob = osb.tile([128, D], f16, tag="ob", name="ob")
                for c in range(D // NCH):
                    pp = psq.tile([128, NCH], f32, tag="psq", name="pf")
                    nc.tensor.matmul(
                        pp[:], at[:, 1, ts(t, 128)], wos[:, 1, ts(c, NCH)],
                        start=True, stop=True,
                    )
                    nc.vector.tensor_add(
                        ob[:, ts(c, NCH)], pp[:], obpre[:, t - 12, ts(c, NCH)]
                    )
                nc.sync.dma_start(out[ts(t, 128), :], ob[:])
            return go

        def make_norm(p, ic, accs, dn_dve=False):
            """Softmax normalization for group (p, ic): attnT = num/denom.
            Reads the SBUF-staged copy of the PV accumulators (PSUM direct on
            the final group), so the chain runs off the critical path;
            emitted a few iterations into the NEXT group. (NB: Pool-engine
            offload was tried and reverted -- partition_broadcast lives in a
            separate GpSimd ucode library and each use costs a ~6us
            UNLOAD_LIB/LOAD_LIB pair.)"""
            def go():
                for hh in range(2):
                    sb = accs[hh]
                    # custom-DVE ops drop the partition base offset; stage
                    # the denominator row to SBUF partition 0 (DMA shifts
                    # partitions; keeps DVE readers off the staged tile --
                    # except on the final group, where latency wins)
                    dn = msc.tile([1, NCH], f32, tag="dn", bufs=2)
                    if dn_dve:
                        nc.vector.tensor_copy(dn[:], sb[DH: DH + 1, :])
                    else:
                        nc.sync.dma_start(dn[:], sb[DH: DH + 1, :])
                    rc = msc.tile([1, NCH], f32, tag="rc", bufs=2)
                    nc.vector.reciprocal_approx_fast(rc[:], dn[:])
                    rcr = msc.tile([1, NCH], f16, tag="rcr", bufs=2)
                    nc.vector.tensor_copy(rcr[:], rc[:])  # round to f16
                    # PE broadcast: a stride-0 DMA (single-partition reread)
                    # and Pool partition_broadcast (ucode lib thrash) both
                    # measured slower -- the systolic array wins here
                    pb = psq.tile([64, NCH], f32, tag="psq", name="pb")
                    nc.tensor.matmul(
                        pb[:], ones16[:, 0:64], rcr[:], start=True, stop=True
                    )
                    bsb = msc.tile([64, NCH], f32, tag="bsb")
                    nc.vector.tensor_copy(bsb[:], pb[:])
                    dst_sl = ts(ic, NCH)
                    if hh == 0:
                        nc.vector.tensor_mul(
                            at[0:DH, p, dst_sl], sb[0:DH, :], bsb[:]
                        )
                    else:
                        # DVE lanes can't shift partitions; bounce via DMA
                        tmp = msc.tile([DH, NCH], f16, tag="tmp", bufs=2)
                        nc.vector.tensor_mul(tmp[:], sb[0:DH, :], bsb[:])
                        nc.sync.dma_start(at[64:128, p, dst_sl], tmp[:])
            return go

        def make_scores(p, ic):
            def scores(j):
                # disjoint PE row groups (partitions 0-63 / 64-127): the two
                # K=64 matmuls execute concurrently
                sc = pss.tile([128, 2 * NCH], f32, tag="sc", name="sc")
                for hh in range(2):
                    nc.tensor.matmul(
                        sc[:, ts(hh, NCH)],
                        kt[hh * 64: hh * 64 + DH, p, ts(j, 128)],
                        qt[hh * 64: hh * 64 + DH, p, ts(ic, NCH)],
                        start=True, stop=True,
                    )
                return sc
            return scores

        seq = [(p, ic) for p in range(FT) for ic in range(T // NCH)]
        scores_of = {g: make_scores(*g) for g in seq}

        # filler schedule: extras[si] = list of thunks, si = flat step index
        extras = {}
        def add(si, th):
            extras.setdefault(si, []).append(th)

        def add2(si, halves):
            add(si, halves[0])
            add(si + 1, halves[1])

        for j in range(TT):                         # g0: V proj just-in-time
            add(j, v_tile(j))   # fillers precede PV(j) (deferred), in time
        add2(1, qk_chunk(wks, FT, kt, 0, 1))
        add2(5, qk_chunk(wks, FT, kt, 0, 2))
        add2(9, qk_chunk(wks, FT, kt, 0, 3))
        add2(13, qk_chunk(wqs, 0, qt, 0, 1))
        add2(16, qk_chunk(wks, FT, kt, 1, 0))
        add2(18, qk_chunk(wqs, 0, qt, 0, 2))
        add2(20, qk_chunk(wks, FT, kt, 1, 1))
        add2(24, qk_chunk(wks, FT, kt, 1, 2))
        add2(28, qk_chunk(wks, FT, kt, 1, 3))
        add2(34, qk_chunk(wqs, 0, qt, 0, 3))
        add2(38, qk_chunk(wqs, 0, qt, 1, 0))
        add2(42, qk_chunk(wqs, 0, qt, 1, 1))
        add2(52, qk_chunk(wqs, 0, qt, 1, 2))
        add2(56, qk_chunk(wqs, 0, qt, 1, 3))
        for i in range(4):                          # out-proj, one ic behind
            add(80 + 4 + 3 * i, outproj_tile(i))
            add(96 + 4 + 3 * i, outproj_tile(4 + i))
            add(112 + 4 + 3 * i, outproj_tile(8 + i))
        # tiles 12-15 gate the epilogue: their ft=0 halves run early (at[:,0]
        # is final after norm(0,3) in group 4); only the ft=1 matmul + add
        # remains after the last norm
        add(85, outproj_pre(12))
        add(91, outproj_pre(13))
        add(101, outproj_pre(14))
        add(107, outproj_pre(15))

        # ---- prologue: just enough projection for the first group ----
        for th in qk_chunk(wks, FT, kt, 0, 0):
            th()
        for th in qk_chunk(wqs, 0, qt, 0, 0):
            th()

        def stage_accs(accs):
            """Copy finished PV accumulators PSUM->SBUF. The next group's
            first PV matmul (same PSUM banks, bufs=1) then only waits for
            these two cheap DVE copies instead of the full norm chain."""
            sb0 = msc.tile([VW, NCH], f32, tag="sb0", bufs=2)
            sb1 = msc.tile([VW, NCH], f32, tag="sb1", bufs=2)
            nc.vector.tensor_copy(sb0[:], accs[0][:])
            nc.vector.tensor_copy(sb1[:], accs[1][:])
            return (sb0, sb1)

        # ---- flat attention pipeline over all 8 groups ----
        # Step i emits: scores(i+1), act(i+1), fillers(i), PV(i-1). Each ACT
        # conservatively waits on ALL earlier-emitted matmuls, so act(i+1)
        # goes in before the step's fillers and the one-step-deferred PV pair
        # -- the scalar exp stream only ever waits on its own scores, and a
        # group-boundary PV (which waits on the acc staging copies) has two
        # act-periods of slack before it may block the PE queue.
        steps = [(gi, p, ic, j) for gi, (p, ic) in enumerate(seq)
                 for j in range(TT)]
        group_accs = {}

        def emit_act(i):
            g2, p2, ic2, j2 = steps[i]
            sc = scores_of[(p2, ic2)](j2)
            pe = ptp.tile([128, 2 * NCH], f16, tag="pe", name="pe", bufs=3)
            nc.scalar.activation(pe[:], sc[:], Exp, scale=0.125)
            return pe

        def emit_pv(i):
            gi, p, ic, j = steps[i]
            accs = group_accs[gi]
            pe = pes.pop(i)
            for hh in range(2):
                nc.tensor.matmul(
                    accs[hh][:, :],
                    vs[:, j, (2 * p + hh) * VW: (2 * p + hh + 1) * VW],
                    pe[:, ts(hh, NCH)],
                    start=(j == 0), stop=(j == TT - 1),
                )
            if j == TT - 1:
                if gi + 1 < len(seq):
                    # stage + hand off to the next group for normalization
                    add(16 * (gi + 1) + 2, make_norm(p, ic, stage_accs(accs)))
                else:
                    # last group: no successor reuses the PSUM banks, so
                    # normalize straight from PSUM (shortest chain)
                    make_norm(p, ic, accs, dn_dve=True)()

        pes = {0: emit_act(0)}
        for i, (gi, p, ic, j) in enumerate(steps):
            if i + 1 < len(steps):
                pes[i + 1] = emit_act(i + 1)
            if j == 0:
                acc0 = pso.tile([VW, NCH], f32, tag="acc0", name="acc0")
                acc1 = pso.tile([VW, NCH], f32, tag="acc1", name="acc1")
                group_accs[gi] = (acc0, acc1)
            for th in extras.get(i, ()):
                th()
            if i > 0:
                emit_pv(i - 1)
        emit_pv(len(steps) - 1)
        for t in range(12, 16):
            outproj_fin(t)()

    nc.finalize()  # Bacc.compile(): wait legalization, reg alloc, act tables
    return nc


def _get_program():
    global _prog
    if _prog is None:
        _prog = _build()
    return _prog


def kernel(x, W_qkv, b_qkv, W_out, b_out):
    global LAST_RESULT
    from concourse.bass_utils import run_bass_kernel_spmd

    x = np.asarray(x, np.float32)
    W_qkv = np.asarray(W_qkv, np.float32)
    b_qkv = np.asarray(b_qkv, np.float32)
    W_out = np.asarray(W_out, np.float32)
    b_out = np.asarray(b_out, np.float32)

    nc = _get_program()

    def pack_k(a):
        # [D, cols] -> SBUF layout [128, KTN*cols]: row p = concat_k a[k*128+p]
        cols = a.shape[1]
        return np.ascontiguousarray(
            a.reshape(KTN, 128, cols).transpose(1, 0, 2).reshape(128, KTN * cols)
        )

    # x packed per batch as [128, (c, k, NCH)] so each 512-col chunk of all
    # eight k-tiles is one contiguous DMA
    xp = {}
    for b in range(B):
        xT = x[b].T.astype(np.float16)              # [D, T]
        v = xT.reshape(KTN, 128, T // NCH, NCH).transpose(1, 2, 0, 3)
        xp[b] = np.ascontiguousarray(v.reshape(128, -1))

    in_maps = []
    for c in range(NCORES):
        b, g = divmod(c, GROUPS)
        sl = slice(g * F, (g + 1) * F)
        # interleave Wv/bv with [zero-weight, bias=1] columns at h*65+64
        wv_g = W_qkv[:, 2 * D:3 * D][:, sl]
        bv_g = b_qkv[2 * D:3 * D][sl]
        wv_i = np.zeros((D, VF), np.float16)
        bv_i = np.zeros((1, VF), np.float16)
        for h in range(HPC):
            wv_i[:, h * VW: h * VW + DH] = wv_g[:, h * DH:(h + 1) * DH]
            bv_i[0, h * VW: h * VW + DH] = bv_g[h * DH:(h + 1) * DH]
            bv_i[0, h * VW + DH] = 1.0
        mb = np.concatenate([
            b_qkv[0 * D:1 * D][sl].reshape(FT, 128).T,
            b_qkv[1 * D:2 * D][sl].reshape(FT, 128).T,
        ], axis=1)                                   # [128, 2*FT] f32
        in_maps.append({
            "xTp": xp[b],
            "wq": pack_k(W_qkv[:, 0 * D:1 * D][:, sl].astype(np.float16)),
            "wk": pack_k(W_qkv[:, 1 * D:2 * D][:, sl].astype(np.float16)),
            "wv": pack_k(wv_i),
            "mb": np.ascontiguousarray(mb),
            "bv": bv_i,
            "wo": np.ascontiguousarray(
                W_out[sl, :].astype(np.float16).reshape(FT, 128, D)
                .transpose(1, 0, 2).reshape(128, FT * D)
            ),
        })

    kw = {}
    if os.environ.get("KERNEL_TRACE") == "1":
        kw["trace"] = True
    res = run_bass_kernel_spmd(nc, in_maps, core_ids=list(range(NCORES)), **kw)
    LAST_RESULT = res

    out = np.empty((B, T, D), np.float32)
    for b in range(B):
        acc = res.results[GROUPS * b]["out"].astype(np.float32)
        for g in range(1, GROUPS):
            acc = acc + res.results[GROUPS * b + g]["out"]
        out[b] = acc + b_out
    return out



# revision 35
# speedup vs baseline: 1.1521x; 1.0030x over previous
"""Multi-head attention (B=2, T=2048, D=1024, H=16, Dh=64) on 8 TRN2 NeuronCores.

Sharding: core c = 4*b + g  ->  batch b in {0,1}, head-group g in {0..3}
(4 heads per core: data parallel on batch, tensor parallel on heads).
Each core computes, for its batch element and its 4 heads:

  Q.T/K.T = Wq/k_shard.T @ x.T + b      [256, 2048]  (head-dim on partitions)
  V'      = x @ Wv_interleaved + b      [2048, 260]  ([V_h | 1] per head)
  per head pair (2p, 2p+1), per 512-wide i-chunk:
    S.T   = K_h Q_h.T                   (two K=64 matmuls on disjoint PE
                                         row groups -> run concurrently)
    P.T   = exp(S.T / 8)                (no max-subtraction: |S|/8 <~ 6)
    acc   = [V_h | 1].T @ P.T           [65, 512]  row 64 = softmax denom
    attnT = acc[:64] * (1/acc[64])
  partial = attnT.T @ Wout_shard        [2048, 1024]

The partial sum over the 4 head groups plus b_out is done on the host
("all-reduce after out_proj"), as is the batch unshard.

Matmuls run in fp16 (1 pass/row on the PE; fp32 PSUM accumulate).
The kernel is organized as one flat software pipeline over the 8
attention groups: the scalar engine's exp stream never breaks; V/Q/K
projection chunks, out-projection tiles, and softmax normalizations are
emitted as filler work inside the attention j-loops.
"""

import os
import numpy as np

B, T, D = 2, 2048, 1024
H, DH = 16, 64
NCORES, GROUPS = 8, 4
HPC = H // GROUPS        # 4 heads per core
F = HPC * DH             # 256 features per core
FT = F // 128            # 2 feature tiles / head pairs
KTN = D // 128           # 8 contraction tiles
TT = T // 128            # 16 token tiles
NCH = 512                # matmul free-dim chunk
VW = DH + 1              # 65: V plus ones column
VF = HPC * VW            # 260: interleaved [V_h | 1] x 4 heads

_prog = None
LAST_RESULT = None


def _build():
    from contextlib import ExitStack

    import concourse.mybir as mybir
    import concourse.tile as tile
    from concourse import bacc
    from concourse.bass import ts, AP

    f32 = mybir.dt.float32
    f32r = mybir.dt.float32r
    f16 = mybir.dt.float16
    Exp = mybir.ActivationFunctionType.Exp

    nc = bacc.Bacc()
    # All inputs arrive host-packed in SBUF layout (partition-major), so
    # each load is ONE dma_start: the Sync engine's ~600ns per-dispatch
    # cost, not bandwidth, dominates the pipeline fill.
    TC = T // NCH  # 4 column chunks of x
    xTp = nc.dram_tensor("xTp", [128, TC * KTN * NCH], f16, kind="ExternalInput")
    wq = nc.dram_tensor("wq", [128, KTN * F], f16, kind="ExternalInput")
    wk = nc.dram_tensor("wk", [128, KTN * F], f16, kind="ExternalInput")
    # wv/bv come pre-interleaved from the host: column h*65+64 is a zero
    # weight column whose bias is 1.0, producing the [V_h | 1] layout that
    # supplies the softmax-denominator row of the PV matmul for free.
    wv = nc.dram_tensor("wv", [128, KTN * VF], f16, kind="ExternalInput")
    mb = nc.dram_tensor("mb", [128, 2 * FT], f32, kind="ExternalInput")
    bv = nc.dram_tensor("bv", [1, VF], f16, kind="ExternalInput")
    wo = nc.dram_tensor("wo", [128, FT * D], f16, kind="ExternalInput")
    out = nc.dram_tensor("out", [T, D], f16, kind="ExternalOutput")

    with ExitStack() as ctx:
        tc = ctx.enter_context(tile.TileContext(nc))
        pers = ctx.enter_context(tc.tile_pool(name="pers", bufs=1))
        ptp = ctx.enter_context(tc.tile_pool(name="ptp", bufs=2))
        osb = ctx.enter_context(tc.tile_pool(name="osb", bufs=2))
        msc = ctx.enter_context(tc.tile_pool(name="msc", bufs=2))
        psq = ctx.enter_context(tc.tile_pool(name="psq", bufs=2, space="PSUM"))
        pss = ctx.enter_context(tc.tile_pool(name="pss", bufs=2, space="PSUM"))
        pso = ctx.enter_context(tc.tile_pool(name="pso", bufs=1, space="PSUM"))

        xt = pers.tile([128, TC, KTN, NCH], f16, tag="xt")
        wqs = pers.tile([128, KTN, F], f16, tag="wqs")
        wks = pers.tile([128, KTN, F], f16, tag="wks")
        wvs = pers.tile([128, KTN, VF], f16, tag="wvs")
        mbs = pers.tile([128, 2 * FT], f32, tag="mbs")  # [bq ft0, bq ft1, bk ft0, bk ft1]
        bvr = pers.tile([1, VF], f16, tag="bvr")
        ones_f = pers.tile([1, 128], f32, tag="ones_f")
        ones16 = pers.tile([1, 128], f16, tag="ones16")
        wos = pers.tile([128, FT, D], f16, tag="wos")
        qt = pers.tile([128, FT, T], f16, tag="qt")
        kt = pers.tile([128, FT, T], f16, tag="kt")
        vs = pers.tile([128, TT, VF], f16, tag="vs")
        at = pers.tile([128, FT, T], f16, tag="at")

        # ISA memset can't target f16; memset f32 then copy-convert
        nc.vector.memset(ones_f[:], 1.0)
        nc.vector.tensor_copy(ones16[:], ones_f[:])

        # DMA dispatch order follows first-use: wk + first x column-chunk
        # unblock the prologue K-projection; the rest streams in behind the
        # pipeline. One dispatch per logical tensor (chunk).
        # wk + first x chunk in halves: the prologue K-projection's first
        # matmuls start as soon as the first halves land
        nc.sync.dma_start(wks[:, 0:KTN // 2], wk[:, 0:(KTN // 2) * F])
        nc.sync.dma_start(xt[:, 0, 0:KTN // 2], xTp[:, 0:(KTN // 2) * NCH])
        nc.sync.dma_start(wks[:, KTN // 2:], wk[:, (KTN // 2) * F:])
        nc.sync.dma_start(
            xt[:, 0, KTN // 2:], xTp[:, (KTN // 2) * NCH:KTN * NCH]
        )
        nc.sync.dma_start(wqs[:], wq[:])
        nc.sync.dma_start(mbs[:], mb[:])
        nc.sync.dma_start(bvr[:], bv[:])
        nc.sync.dma_start(wvs[:], wv[:])
        for c in range(1, TC):
            nc.sync.dma_start(xt[:, c], xTp[:, ts(c, KTN * NCH)])
        nc.sync.dma_start(wos[:], wo[:])

        # ---- deferred work units (emitted inside attention j-loops) ----
        def qk_chunk(wsb, bix, dst, ft, c):
            """Returns the two 4-matmul halves of one projection chunk, so a
            chunk spreads over two adjacent pipeline steps and never delays a
            step's scores by more than ~0.9us of PE time."""
            state = {}

            def half0():
                state["ps"] = psq.tile([128, NCH], f32, tag="psq", name="ps")
                for k in range(KTN // 2):
                    nc.tensor.matmul(
                        state["ps"][:],
                        wsb[:, k, ts(ft, 128)],
                        xt[:, c, k, :],
                        start=(k == 0), stop=False,
                    )

            def half1():
                ps = state["ps"]
                for k in range(KTN // 2, KTN):
                    nc.tensor.matmul(
                        ps[:],
                        wsb[:, k, ts(ft, 128)],
                        xt[:, c, k, :],
                        start=False, stop=(k == KTN - 1),
                    )
                nc.vector.tensor_scalar_add(
                    dst[:, ft, ts(c, NCH)], ps[:], mbs[:, bix + ft: bix + ft + 1]
                )
            return half0, half1

        def v_tile(t):
            def go():
                pv = psq.tile([128, VF], f32, tag="psq", name="pv")
                for k in range(KTN):
                    nc.tensor.matmul(
                        pv[:], xt[:, t // 4, k, ts(t % 4, 128)], wvs[:, k, :],
                        start=(k == 0), stop=False,
                    )
                # bias via ones-row (also writes the denominator 1.0 cols)
                nc.tensor.matmul(
                    pv[:], ones16[:, 0:128], bvr[:], start=False, stop=True
                )
                nc.vector.tensor_copy(vs[:, t, :], pv[:])
            return go

        def outproj_tile(t):
            def go():
                # partials leave as f16: halves the 8MB output drain DMA
                ob = osb.tile([128, D], f16, tag="ob", name="ob")
                for c in range(D // NCH):
                    pp = psq.tile([128, NCH], f32, tag="psq", name="pp")
                    for ft in range(FT):
                        nc.tensor.matmul(
                            pp[:],
                            at[:, ft, ts(t, 128)],
                            wos[:, ft, ts(c, NCH)],
                            start=(ft == 0), stop=(ft == FT - 1),
                        )
                    nc.vector.tensor_copy(ob[:, ts(c, NCH)], pp[:])
                nc.sync.dma_start(out[ts(t, 128), :], ob[:])
            return go

        obpre = pers.tile([128, 4, D], f32, tag="obpre")  # ft=0 partials, t=12..15

        def outproj_pre(t):
            def go():
                for c in range(D // NCH):
                    pp = psq.tile([128, NCH], f32, tag="psq", name="pq")
                    nc.tensor.matmul(
                        pp[:], at[:, 0, ts(t, 128)], wos[:, 0, ts(c, NCH)],
                        start=True, stop=True,
                    )
                    nc.vector.tensor_copy(obpre[:, t - 12, ts(c, NCH)], pp[:])
            return go

        def outproj_fin(t):
            def go():
                ob = osb.tile([128, D], f16, tag="ob", name="ob")
                for c in range(D // NCH):
                    pp = psq.tile([128, NCH], f32, tag="psq", name="pf")
                    nc.tensor.matmul(
                        pp[:], at[:, 1, ts(t, 128)], wos[:, 1, ts(c, NCH)],
                        start=True, stop=True,
                    )
                    nc.vector.tensor_add(
                        ob[:, ts(c, NCH)], pp[:], obpre[:, t - 12, ts(c, NCH)]
                    )
                nc.sync.dma_start(out[ts(t, 128), :], ob[:])
            return go

        def make_norm(p, ic, accs, dn_dve=False):
            """Softmax normalization for group (p, ic): attnT = num/denom.
            Reads the SBUF-staged copy of the PV accumulators (PSUM direct on
            the final group), so the chain runs off the critical path;
            emitted a few iterations into the NEXT group. (NB: Pool-engine
            offload was tried and reverted -- partition_broadcast lives in a
            separate GpSimd ucode library and each use costs a ~6us
            UNLOAD_LIB/LOAD_LIB pair.)"""
            def go():
                for hh in range(2):
                    sb = accs[hh]
                    # custom-DVE ops drop the partition base offset; stage
                    # the denominator row to SBUF partition 0 (DMA shifts
                    # partitions; keeps DVE readers off the staged tile --
                    # except on the final group, where latency wins)
                    dn = msc.tile([1, NCH], f32, tag="dn", bufs=2)
                    if dn_dve:
                        nc.vector.tensor_copy(dn[:], sb[DH: DH + 1, :])
                    else:
                        nc.sync.dma_start(dn[:], sb[DH: DH + 1, :])
                    rc = msc.tile([1, NCH], f32, tag="rc", bufs=2)
                    nc.vector.reciprocal_approx_fast(rc[:], dn[:])
                    rcr = msc.tile([1, NCH], f16, tag="rcr", bufs=2)
                    nc.vector.tensor_copy(rcr[:], rc[:])  # round to f16
                    # PE broadcast: a stride-0 DMA (single-partition reread)
                    # and Pool partition_broadcast (ucode lib thrash) both
                    # measured slower -- the systolic array wins here
                    pb = psq.tile([64, NCH], f32, tag="psq", name="pb")
                    nc.tensor.matmul(
                        pb[:], ones16[:, 0:64], rcr[:], start=True, stop=True
                    )
                    bsb = msc.tile([64, NCH], f32, tag="bsb")
                    nc.vector.tensor_copy(bsb[:], pb[:])
                    dst_sl = ts(ic, NCH)
                    if hh == 0:
                        nc.vector.tensor_mul(
                            at[0:DH, p, dst_sl], sb[0:DH, :], bsb[:]
                        )
                    else:
                        # DVE lanes can't shift partitions; bounce via DMA
                        tmp = msc.tile([DH, NCH], f16, tag="tmp", bufs=2)
                        nc.vector.tensor_mul(tmp[:], sb[0:DH, :], bsb[:])
                        nc.sync.dma_start(at[64:128, p, dst_sl], tmp[:])
            return go

        def make_scores(p, ic):
            def scores(j):
                # disjoint PE row groups (partitions 0-63 / 64-127): the two
                # K=64 matmuls execute concurrently
                sc = pss.tile([128, 2 * NCH], f32, tag="sc", name="sc")
                for hh in range(2):
                    nc.tensor.matmul(
                        sc[:, ts(hh, NCH)],
                        kt[hh * 64: hh * 64 + DH, p, ts(j, 128)],
                        qt[hh * 64: hh * 64 + DH, p, ts(ic, NCH)],
                        start=True, stop=True,
                    )
                return sc
            return scores

        seq = [(p, ic) for p in range(FT) for ic in range(T // NCH)]
        scores_of = {g: make_scores(*g) for g in seq}

        # filler schedule: extras[si] = list of thunks, si = flat step index
        extras = {}
        def add(si, th):
            extras.setdefault(si, []).append(th)

        def add2(si, halves):
            add(si, halves[0])
            add(si + 1, halves[1])

        for j in range(TT):                         # g0: V proj just-in-time
            add(j, v_tile(j))   # fillers precede PV(j) (deferred), in time
        # group-boundary steps (si = 16g, 16g+1) get filler matmuls: the
        # deferred PV(old,15) + acc staging copies would otherwise leave the
        # PE briefly starved there
        add2(1, qk_chunk(wks, FT, kt, 0, 1))
        add2(5, qk_chunk(wks, FT, kt, 0, 2))
        add2(9, qk_chunk(wks, FT, kt, 0, 3))
        add2(13, qk_chunk(wqs, 0, qt, 0, 1))
        add2(16, qk_chunk(wks, FT, kt, 1, 0))
        add2(18, qk_chunk(wqs, 0, qt, 0, 2))
        add2(20, qk_chunk(wks, FT, kt, 1, 1))
        add2(24, qk_chunk(wks, FT, kt, 1, 2))
        add2(28, qk_chunk(wks, FT, kt, 1, 3))
        add2(32, qk_chunk(wqs, 0, qt, 0, 3))
        add2(36, qk_chunk(wqs, 0, qt, 1, 0))
        add2(42, qk_chunk(wqs, 0, qt, 1, 1))
        add2(48, qk_chunk(wqs, 0, qt, 1, 2))
        add2(64, qk_chunk(wqs, 0, qt, 1, 3))
        for i in range(4):                          # out-proj, one ic behind
            add(80 + 4 + 3 * i, outproj_tile(i))
            add(96 + 4 + 3 * i, outproj_tile(4 + i))
            add(112 + 4 + 3 * i, outproj_tile(8 + i))
        # tiles 12-15 gate the epilogue: their ft=0 halves run early (at[:,0]
        # is final after norm(0,3) in group 4); only the ft=1 matmul + add
        # remains after the last norm
        add(80, outproj_pre(12))
        add(85, outproj_pre(13))
        add(96, outproj_pre(14))
        add(112, outproj_pre(15))

        # ---- prologue: just enough projection for the first group ----
        for th in qk_chunk(wks, FT, kt, 0, 0):
            th()
        for th in qk_chunk(wqs, 0, qt, 0, 0):
            th()

        def stage_accs(accs):
            """Copy finished PV accumulators PSUM->SBUF. The next group's
            first PV matmul (same PSUM banks, bufs=1) then only waits for
            these two cheap DVE copies instead of the full norm chain."""
            sb0 = msc.tile([VW, NCH], f32, tag="sb0", bufs=2)
            sb1 = msc.tile([VW, NCH], f32, tag="sb1", bufs=2)
            nc.vector.tensor_copy(sb0[:], accs[0][:])
            nc.vector.tensor_copy(sb1[:], accs[1][:])
            return (sb0, sb1)

        # ---- flat attention pipeline over all 8 groups ----
        # Step i emits: scores(i+1), act(i+1), fillers(i), PV(i-1). Each ACT
        # conservatively waits on ALL earlier-emitted matmuls, so act(i+1)
        # goes in before the step's fillers and the one-step-deferred PV pair
        # -- the scalar exp stream only ever waits on its own scores, and a
        # group-boundary PV (which waits on the acc staging copies) has two
        # act-periods of slack before it may block the PE queue.
        steps = [(gi, p, ic, j) for gi, (p, ic) in enumerate(seq)
                 for j in range(TT)]
        group_accs = {}

        def emit_act(i):
            g2, p2, ic2, j2 = steps[i]
            sc = scores_of[(p2, ic2)](j2)
            pe = ptp.tile([128, 2 * NCH], f16, tag="pe", name="pe", bufs=3)
            nc.scalar.activation(pe[:], sc[:], Exp, scale=0.125)
            return pe

        def emit_pv(i):
            gi, p, ic, j = steps[i]
            accs = group_accs[gi]
            pe = pes.pop(i)
            for hh in range(2):
                nc.tensor.matmul(
                    accs[hh][:, :],
                    vs[:, j, (2 * p + hh) * VW: (2 * p + hh + 1) * VW],
                    pe[:, ts(hh, NCH)],
                    start=(j == 0), stop=(j == TT - 1),
                )
            if j == TT - 1:
                if gi + 1 < len(seq):
                    # stage + hand off to the next group for normalization
                    add(16 * (gi + 1) + 2, make_norm(p, ic, stage_accs(accs)))
                else:
                    # last group: no successor reuses the PSUM banks, so
                    # normalize straight from PSUM (shortest chain)
                    make_norm(p, ic, accs, dn_dve=True)()

        pes = {0: emit_act(0)}
        for i, (gi, p, ic, j) in enumerate(steps):
            if i + 1 < len(steps):
                pes[i + 1] = emit_act(i + 1)
            if j == 0:
                acc0 = pso.tile([VW, NCH], f32, tag="acc0", name="acc0")
                acc1 = pso.tile([VW, NCH], f32, tag="acc1", name="acc1")
                group_accs[gi] = (acc0, acc1)
            for th in extras.get(i, ()):
                th()
            if i > 0:
                emit_pv(i - 1)
        emit_pv(len(steps) - 1)
        for t in range(12, 16):
            outproj_fin(t)()

    nc.finalize()  # Bacc.compile(): wait legalization, reg alloc, act tables
    return nc


def _get_program():
    global _prog
    if _prog is None:
        _prog = _build()
    return _prog


def kernel(x, W_qkv, b_qkv, W_out, b_out):
    global LAST_RESULT
    from concourse.bass_utils import run_bass_kernel_spmd

    x = np.asarray(x, np.float32)
    W_qkv = np.asarray(W_qkv, np.float32)
    b_qkv = np.asarray(b_qkv, np.float32)
    W_out = np.asarray(W_out, np.float32)
    b_out = np.asarray(b_out, np.float32)

    nc = _get_program()

    def pack_k(a):
        # [D, cols] -> SBUF layout [128, KTN*cols]: row p = concat_k a[k*128+p]
        cols = a.shape[1]
        return np.ascontiguousarray(
            a.reshape(KTN, 128, cols).transpose(1, 0, 2).reshape(128, KTN * cols)
        )

    # x packed per batch as [128, (c, k, NCH)] so each 512-col chunk of all
    # eight k-tiles is one contiguous DMA
    xp = {}
    for b in range(B):
        xT = x[b].T.astype(np.float16)              # [D, T]
        v = xT.reshape(KTN, 128, T // NCH, NCH).transpose(1, 2, 0, 3)
        xp[b] = np.ascontiguousarray(v.reshape(128, -1))

    in_maps = []
    for c in range(NCORES):
        b, g = divmod(c, GROUPS)
        sl = slice(g * F, (g + 1) * F)
        # interleave Wv/bv with [zero-weight, bias=1] columns at h*65+64
        wv_g = W_qkv[:, 2 * D:3 * D][:, sl]
        bv_g = b_qkv[2 * D:3 * D][sl]
        wv_i = np.zeros((D, VF), np.float16)
        bv_i = np.zeros((1, VF), np.float16)
        for h in range(HPC):
            wv_i[:, h * VW: h * VW + DH] = wv_g[:, h * DH:(h + 1) * DH]
            bv_i[0, h * VW: h * VW + DH] = bv_g[h * DH:(h + 1) * DH]
            bv_i[0, h * VW + DH] = 1.0
        mb = np.concatenate([
            b_qkv[0 * D:1 * D][sl].reshape(FT, 128).T,
            b_qkv[1 * D:2 * D][sl].reshape(FT, 128).T,
        ], axis=1)                                   # [128, 2*FT] f32
        in_maps.append({
            "xTp": xp[b],
            "wq": pack_k(W_qkv[:, 0 * D:1 * D][:, sl].astype(np.float16)),
            "wk": pack_k(W_qkv[:, 1 * D:2 * D][:, sl].astype(np.float16)),
            "wv": pack_k(wv_i),
            "mb": np.ascontiguousarray(mb),
            "bv": bv_i,
            "wo": np.ascontiguousarray(
                W_out[sl, :].astype(np.float16).reshape(FT, 128, D)
                .transpose(1, 0, 2).reshape(128, FT * D)
            ),
        })

    kw = {}
    if os.environ.get("KERNEL_TRACE") == "1":
        kw["trace"] = True
    res = run_bass_kernel_spmd(nc, in_maps, core_ids=list(range(NCORES)), **kw)
    LAST_RESULT = res

    out = np.empty((B, T, D), np.float32)
    for b in range(B):
        acc = res.results[GROUPS * b]["out"].astype(np.float32)
        for g in range(1, GROUPS):
            acc = acc + res.results[GROUPS * b + g]["out"]
        out[b] = acc + b_out
    return out



# revision 39
# speedup vs baseline: 1.1582x; 1.0053x over previous
"""Multi-head attention (B=2, T=2048, D=1024, H=16, Dh=64) on 8 TRN2 NeuronCores.

Sharding: core c = 4*b + g  ->  batch b in {0,1}, head-group g in {0..3}
(4 heads per core: data parallel on batch, tensor parallel on heads).
Each core computes, for its batch element and its 4 heads:

  Q.T/K.T = Wq/k_shard.T @ x.T + b      [256, 2048]  (head-dim on partitions)
  V'      = x @ Wv_interleaved + b      [2048, 260]  ([V_h | 1] per head)
  per head pair (2p, 2p+1), per 512-wide i-chunk:
    S.T   = K_h Q_h.T                   (two K=64 matmuls on disjoint PE
                                         row groups -> run concurrently)
    P.T   = exp(S.T / 8)                (no max-subtraction: |S|/8 <~ 6)
    acc   = [V_h | 1].T @ P.T           [65, 512]  row 64 = softmax denom
    attnT = acc[:64] * (1/acc[64])
  partial = attnT.T @ Wout_shard        [2048, 1024]

The partial sum over the 4 head groups plus b_out is done on the host
("all-reduce after out_proj"), as is the batch unshard.

Matmuls run in fp16 (1 pass/row on the PE; fp32 PSUM accumulate).
The kernel is organized as one flat software pipeline over the 8
attention groups: the scalar engine's exp stream never breaks; V/Q/K
projection chunks, out-projection tiles, and softmax normalizations are
emitted as filler work inside the attention j-loops.
"""

import os
import numpy as np

B, T, D = 2, 2048, 1024
H, DH = 16, 64
NCORES, GROUPS = 8, 4
HPC = H // GROUPS        # 4 heads per core
F = HPC * DH             # 256 features per core
FT = F // 128            # 2 feature tiles / head pairs
KTN = D // 128           # 8 contraction tiles
TT = T // 128            # 16 token tiles
NCH = 512                # matmul free-dim chunk
VW = DH + 1              # 65: V plus ones column
VF = HPC * VW            # 260: interleaved [V_h | 1] x 4 heads

_prog = None
LAST_RESULT = None


def _build():
    from contextlib import ExitStack

    import concourse.mybir as mybir
    import concourse.tile as tile
    from concourse import bacc
    from concourse.bass import ts, AP

    f32 = mybir.dt.float32
    f32r = mybir.dt.float32r
    f16 = mybir.dt.float16
    Exp = mybir.ActivationFunctionType.Exp

    nc = bacc.Bacc()
    # All inputs arrive host-packed in SBUF layout (partition-major), so
    # each load is ONE dma_start: the Sync engine's ~600ns per-dispatch
    # cost, not bandwidth, dominates the pipeline fill.
    TC = T // NCH  # 4 column chunks of x
    xTp = nc.dram_tensor("xTp", [128, TC * KTN * NCH], f16, kind="ExternalInput")
    wq = nc.dram_tensor("wq", [128, KTN * F], f16, kind="ExternalInput")
    wk = nc.dram_tensor("wk", [128, KTN * F], f16, kind="ExternalInput")
    # wv/bv come pre-interleaved from the host: column h*65+64 is a zero
    # weight column whose bias is 1.0, producing the [V_h | 1] layout that
    # supplies the softmax-denominator row of the PV matmul for free.
    wv = nc.dram_tensor("wv", [128, KTN * VF], f16, kind="ExternalInput")
    mb = nc.dram_tensor("mb", [128, 2 * FT], f32, kind="ExternalInput")
    bv = nc.dram_tensor("bv", [1, VF], f16, kind="ExternalInput")
    wo = nc.dram_tensor("wo", [128, FT * D], f16, kind="ExternalInput")
    out = nc.dram_tensor("out", [T, D], f16, kind="ExternalOutput")

    with ExitStack() as ctx:
        tc = ctx.enter_context(tile.TileContext(nc))
        pers = ctx.enter_context(tc.tile_pool(name="pers", bufs=1))
        ptp = ctx.enter_context(tc.tile_pool(name="ptp", bufs=2))
        osb = ctx.enter_context(tc.tile_pool(name="osb", bufs=2))
        msc = ctx.enter_context(tc.tile_pool(name="msc", bufs=2))
        psq = ctx.enter_context(tc.tile_pool(name="psq", bufs=2, space="PSUM"))
        pss = ctx.enter_context(tc.tile_pool(name="pss", bufs=2, space="PSUM"))
        pso = ctx.enter_context(tc.tile_pool(name="pso", bufs=1, space="PSUM"))

        xt = pers.tile([128, TC, KTN, NCH], f16, tag="xt")
        wqs = pers.tile([128, KTN, F], f16, tag="wqs")
        wks = pers.tile([128, KTN, F], f16, tag="wks")
        wvs = pers.tile([128, KTN, VF], f16, tag="wvs")
        mbs = pers.tile([128, 2 * FT], f32, tag="mbs")  # [bq ft0, bq ft1, bk ft0, bk ft1]
        bvr = pers.tile([1, VF], f16, tag="bvr")
        ones_f = pers.tile([1, 128], f32, tag="ones_f")
        ones16 = pers.tile([1, 128], f16, tag="ones16")
        wos = pers.tile([128, FT, D], f16, tag="wos")
        qt = pers.tile([128, FT, T], f16, tag="qt")
        kt = pers.tile([128, FT, T], f16, tag="kt")
        vs = pers.tile([128, TT, VF], f16, tag="vs")
        at = pers.tile([128, FT, T], f16, tag="at")

        # ISA memset can't target f16; memset f32 then copy-convert
        nc.vector.memset(ones_f[:], 1.0)
        nc.vector.tensor_copy(ones16[:], ones_f[:])

        # DMA dispatch order follows first-use: wk + first x column-chunk
        # unblock the prologue K-projection; the rest streams in behind the
        # pipeline. One dispatch per logical tensor (chunk).
        # wk/wq + first x chunk in halves, ordered so each prologue
        # projection half's last dependency lands as late as possible after
        # the prior half's compute: the first scores fire ~4us earlier
        KH = KTN // 2
        nc.sync.dma_start(wks[:, 0:KH], wk[:, 0:KH * F])
        nc.sync.dma_start(xt[:, 0, 0:KH], xTp[:, 0:KH * NCH])
        nc.sync.dma_start(wqs[:, 0:KH], wq[:, 0:KH * F])
        nc.sync.dma_start(xt[:, 0, KH:], xTp[:, KH * NCH:KTN * NCH])
        nc.sync.dma_start(wks[:, KH:], wk[:, KH * F:])
        nc.sync.dma_start(wqs[:, KH:], wq[:, KH * F:])
        nc.sync.dma_start(mbs[:], mb[:])
        nc.sync.dma_start(bvr[:], bv[:])
        nc.sync.dma_start(wvs[:], wv[:])
        for c in range(1, TC):
            nc.sync.dma_start(xt[:, c], xTp[:, ts(c, KTN * NCH)])
        nc.sync.dma_start(wos[:], wo[:])

        # ---- deferred work units (emitted inside attention j-loops) ----
        def qk_chunk(wsb, bix, dst, ft, c):
            """Returns the two 4-matmul halves of one projection chunk, so a
            chunk spreads over two adjacent pipeline steps and never delays a
            step's scores by more than ~0.9us of PE time."""
            state = {}

            def half0():
                state["ps"] = psq.tile([128, NCH], f32, tag="psq", name="ps")
                for k in range(KTN // 2):
                    nc.tensor.matmul(
                        state["ps"][:],
                        wsb[:, k, ts(ft, 128)],
                        xt[:, c, k, :],
                        start=(k == 0), stop=False,
                    )

            def half1():
                ps = state["ps"]
                for k in range(KTN // 2, KTN):
                    nc.tensor.matmul(
                        ps[:],
                        wsb[:, k, ts(ft, 128)],
                        xt[:, c, k, :],
                        start=False, stop=(k == KTN - 1),
                    )
                nc.vector.tensor_scalar_add(
                    dst[:, ft, ts(c, NCH)], ps[:], mbs[:, bix + ft: bix + ft + 1]
                )
            return half0, half1

        def v_tile(t):
            def go():
                pv = psq.tile([128, VF], f32, tag="psq", name="pv")
                for k in range(KTN):
                    nc.tensor.matmul(
                        pv[:], xt[:, t // 4, k, ts(t % 4, 128)], wvs[:, k, :],
                        start=(k == 0), stop=False,
                    )
                # bias via ones-row (also writes the denominator 1.0 cols)
                nc.tensor.matmul(
                    pv[:], ones16[:, 0:128], bvr[:], start=False, stop=True
                )
                nc.vector.tensor_copy(vs[:, t, :], pv[:])
            return go

        def outproj_tile(t):
            def go():
                # partials leave as f16: halves the 8MB output drain DMA
                ob = osb.tile([128, D], f16, tag="ob", name="ob")
                for c in range(D // NCH):
                    pp = psq.tile([128, NCH], f32, tag="psq", name="pp")
                    for ft in range(FT):
                        nc.tensor.matmul(
                            pp[:],
                            at[:, ft, ts(t, 128)],
                            wos[:, ft, ts(c, NCH)],
                            start=(ft == 0), stop=(ft == FT - 1),
                        )
                    nc.vector.tensor_copy(ob[:, ts(c, NCH)], pp[:])
                nc.sync.dma_start(out[ts(t, 128), :], ob[:])
            return go

        obpre = pers.tile([128, 4, D], f32, tag="obpre")  # ft=0 partials, t=12..15

        def outproj_pre(t):
            def go():
                for c in range(D // NCH):
                    pp = psq.tile([128, NCH], f32, tag="psq", name="pq")
                    nc.tensor.matmul(
                        pp[:], at[:, 0, ts(t, 128)], wos[:, 0, ts(c, NCH)],
                        start=True, stop=True,
                    )
                    nc.vector.tensor_copy(obpre[:, t - 12, ts(c, NCH)], pp[:])
            return go

        def outproj_fin(t):
            def go():
                ob = osb.tile([128, D], f16, tag="ob", name="ob")
                for c in range(D // NCH):
                    pp = psq.tile([128, NCH], f32, tag="psq", name="pf")
                    nc.tensor.matmul(
                        pp[:], at[:, 1, ts(t, 128)], wos[:, 1, ts(c, NCH)],
                        start=True, stop=True,
                    )
                    nc.vector.tensor_add(
                        ob[:, ts(c, NCH)], pp[:], obpre[:, t - 12, ts(c, NCH)]
                    )
                nc.sync.dma_start(out[ts(t, 128), :], ob[:])
            return go

        def make_norm(p, ic, accs, dn_dve=False):
            """Softmax normalization for group (p, ic): attnT = num/denom.
            Reads the SBUF-staged copy of the PV accumulators (PSUM direct on
            the final group), so the chain runs off the critical path;
            emitted a few iterations into the NEXT group. (NB: Pool-engine
            offload was tried and reverted -- partition_broadcast lives in a
            separate GpSimd ucode library and each use costs a ~6us
            UNLOAD_LIB/LOAD_LIB pair.)"""
            def go():
                # final group: odd head first, so its at-bounce DMA flies
                # while the even head's DVE chain runs
                for hh in ((1, 0) if dn_dve else (0, 1)):
                    sb = accs[hh]
                    # custom-DVE ops drop the partition base offset; stage
                    # the denominator row to SBUF partition 0 (DMA shifts
                    # partitions; keeps DVE readers off the staged tile --
                    # except on the final group, where latency wins)
                    dn = msc.tile([1, NCH], f32, tag="dn", bufs=2)
                    if dn_dve:
                        nc.vector.tensor_copy(dn[:], sb[DH: DH + 1, :])
                    else:
                        nc.sync.dma_start(dn[:], sb[DH: DH + 1, :])
                    rc = msc.tile([1, NCH], f32, tag="rc", bufs=2)
                    nc.vector.reciprocal_approx_fast(rc[:], dn[:])
                    rcr = msc.tile([1, NCH], f16, tag="rcr", bufs=2)
                    nc.vector.tensor_copy(rcr[:], rc[:])  # round to f16
                    # PE broadcast: a stride-0 DMA (single-partition reread)
                    # and Pool partition_broadcast (ucode lib thrash) both
                    # measured slower -- the systolic array wins here
                    pb = psq.tile([64, NCH], f32, tag="psq", name="pb")
                    nc.tensor.matmul(
                        pb[:], ones16[:, 0:64], rcr[:], start=True, stop=True
                    )
                    bsb = msc.tile([64, NCH], f32, tag="bsb")
                    nc.vector.tensor_copy(bsb[:], pb[:])
                    dst_sl = ts(ic, NCH)
                    if hh == 0:
                        nc.vector.tensor_mul(
                            at[0:DH, p, dst_sl], sb[0:DH, :], bsb[:]
                        )
                    else:
                        # DVE lanes can't shift partitions; bounce via DMA
                        tmp = msc.tile([DH, NCH], f16, tag="tmp", bufs=2)
                        nc.vector.tensor_mul(tmp[:], sb[0:DH, :], bsb[:])
                        nc.sync.dma_start(at[64:128, p, dst_sl], tmp[:])
            return go

        def make_scores(p, ic):
            def scores(j):
                # disjoint PE row groups (partitions 0-63 / 64-127): the two
                # K=64 matmuls execute concurrently
                sc = pss.tile([128, 2 * NCH], f32, tag="sc", name="sc")
                for hh in range(2):
                    nc.tensor.matmul(
                        sc[:, ts(hh, NCH)],
                        kt[hh * 64: hh * 64 + DH, p, ts(j, 128)],
                        qt[hh * 64: hh * 64 + DH, p, ts(ic, NCH)],
                        start=True, stop=True,
                    )
                return sc
            return scores

        seq = [(p, ic) for p in range(FT) for ic in range(T // NCH)]
        scores_of = {g: make_scores(*g) for g in seq}

        # filler schedule: extras[si] = list of thunks, si = flat step index
        extras = {}
        def add(si, th):
            extras.setdefault(si, []).append(th)

        def add2(si, halves):
            add(si, halves[0])
            add(si + 1, halves[1])

        for j in range(TT):                         # g0: V proj just-in-time
            add(j, v_tile(j))   # fillers precede PV(j) (deferred), in time
        # group-boundary steps (si = 16g, 16g+1) get filler matmuls: the
        # deferred PV(old,15) + acc staging copies would otherwise leave the
        # PE briefly starved there
        add2(1, qk_chunk(wks, FT, kt, 0, 1))
        add2(5, qk_chunk(wks, FT, kt, 0, 2))
        add2(9, qk_chunk(wks, FT, kt, 0, 3))
        add2(13, qk_chunk(wqs, 0, qt, 0, 1))
        add2(16, qk_chunk(wks, FT, kt, 1, 0))
        add2(18, qk_chunk(wqs, 0, qt, 0, 2))
        add2(20, qk_chunk(wks, FT, kt, 1, 1))
        add2(24, qk_chunk(wks, FT, kt, 1, 2))
        add2(28, qk_chunk(wks, FT, kt, 1, 3))
        add2(32, qk_chunk(wqs, 0, qt, 0, 3))
        add2(36, qk_chunk(wqs, 0, qt, 1, 0))
        add2(42, qk_chunk(wqs, 0, qt, 1, 1))
        add2(48, qk_chunk(wqs, 0, qt, 1, 2))
        add2(64, qk_chunk(wqs, 0, qt, 1, 3))
        for i in range(4):                          # out-proj, one ic behind
            add(80 + 4 + 3 * i, outproj_tile(i))
            add(96 + 4 + 3 * i, outproj_tile(4 + i))
            add(112 + 4 + 3 * i, outproj_tile(8 + i))
        # tiles 12-15 gate the epilogue: their ft=0 halves run early (at[:,0]
        # is final after norm(0,3) in group 4); only the ft=1 matmul + add
        # remains after the last norm
        add(80, outproj_pre(12))
        add(85, outproj_pre(13))
        add(96, outproj_pre(14))
        add(112, outproj_pre(15))

        # ---- prologue: just enough projection for the first group ----
        # halves interleaved to track the DMA arrival order above
        kt00 = qk_chunk(wks, FT, kt, 0, 0)
        qt00 = qk_chunk(wqs, 0, qt, 0, 0)
        kt00[0]()
        qt00[0]()
        kt00[1]()
        qt00[1]()

        def stage_accs(accs):
            """Copy finished PV accumulators PSUM->SBUF. The next group's
            first PV matmul (same PSUM banks, bufs=1) then only waits for
            these two cheap DVE copies instead of the full norm chain."""
            sb0 = msc.tile([VW, NCH], f32, tag="sb0", bufs=2)
            sb1 = msc.tile([VW, NCH], f32, tag="sb1", bufs=2)
            nc.vector.tensor_copy(sb0[:], accs[0][:])
            nc.vector.tensor_copy(sb1[:], accs[1][:])
            return (sb0, sb1)

        # ---- flat attention pipeline over all 8 groups ----
        # Step i emits: scores(i+1), act(i+1), fillers(i), PV(i-1). Each ACT
        # conservatively waits on ALL earlier-emitted matmuls, so act(i+1)
        # goes in before the step's fillers and the one-step-deferred PV pair
        # -- the scalar exp stream only ever waits on its own scores, and a
        # group-boundary PV (which waits on the acc staging copies) has two
        # act-periods of slack before it may block the PE queue.
        steps = [(gi, p, ic, j) for gi, (p, ic) in enumerate(seq)
                 for j in range(TT)]
        group_accs = {}

        def emit_act(i):
            g2, p2, ic2, j2 = steps[i]
            sc = scores_of[(p2, ic2)](j2)
            pe = ptp.tile([128, 2 * NCH], f16, tag="pe", name="pe", bufs=3)
            nc.scalar.activation(pe[:], sc[:], Exp, scale=0.125)
            return pe

        def emit_pv(i):
            gi, p, ic, j = steps[i]
            accs = group_accs[gi]
            pe = pes.pop(i)
            for hh in range(2):
                nc.tensor.matmul(
                    accs[hh][:, :],
                    vs[:, j, (2 * p + hh) * VW: (2 * p + hh + 1) * VW],
                    pe[:, ts(hh, NCH)],
                    start=(j == 0), stop=(j == TT - 1),
                )
            if j == TT - 1:
                if gi + 1 < len(seq):
                    # stage + hand off to the next group for normalization
                    add(16 * (gi + 1) + 2, make_norm(p, ic, stage_accs(accs)))
                else:
                    # last group: no successor reuses the PSUM banks, so
                    # normalize straight from PSUM (shortest chain)
                    make_norm(p, ic, accs, dn_dve=True)()

        pes = {0: emit_act(0)}
        for i, (gi, p, ic, j) in enumerate(steps):
            if i + 1 < len(steps):
                pes[i + 1] = emit_act(i + 1)
            if j == 0:
                acc0 = pso.tile([VW, NCH], f32, tag="acc0", name="acc0")
                acc1 = pso.tile([VW, NCH], f32, tag="acc1", name="acc1")
                group_accs[gi] = (acc0, acc1)
            for th in extras.get(i, ()):
                th()
            if i > 0:
                emit_pv(i - 1)
        emit_pv(len(steps) - 1)
        for t in range(12, 16):
            outproj_fin(t)()

    nc.finalize()  # Bacc.compile(): wait legalization, reg alloc, act tables
    return nc


def _get_program():
    global _prog
    if _prog is None:
        _prog = _build()
    return _prog


def kernel(x, W_qkv, b_qkv, W_out, b_out):
    global LAST_RESULT
    from concourse.bass_utils import run_bass_kernel_spmd

    x = np.asarray(x, np.float32)
    W_qkv = np.asarray(W_qkv, np.float32)
    b_qkv = np.asarray(b_qkv, np.float32)
    W_out = np.asarray(W_out, np.float32)
    b_out = np.asarray(b_out, np.float32)

    nc = _get_program()

    def pack_k(a):
        # [D, cols] -> SBUF layout [128, KTN*cols]: row p = concat_k a[k*128+p]
        cols = a.shape[1]
        return np.ascontiguousarray(
            a.reshape(KTN, 128, cols).transpose(1, 0, 2).reshape(128, KTN * cols)
        )

    # x packed per batch as [128, (c, k, NCH)] so each 512-col chunk of all
    # eight k-tiles is one contiguous DMA
    xp = {}
    for b in range(B):
        xT = x[b].T.astype(np.float16)              # [D, T]
        v = xT.reshape(KTN, 128, T // NCH, NCH).transpose(1, 2, 0, 3)
        xp[b] = np.ascontiguousarray(v.reshape(128, -1))

    in_maps = []
    for c in range(NCORES):
        b, g = divmod(c, GROUPS)
        sl = slice(g * F, (g + 1) * F)
        # interleave Wv/bv with [zero-weight, bias=1] columns at h*65+64
        wv_g = W_qkv[:, 2 * D:3 * D][:, sl]
        bv_g = b_qkv[2 * D:3 * D][sl]
        wv_i = np.zeros((D, VF), np.float16)
        bv_i = np.zeros((1, VF), np.float16)
        for h in range(HPC):
            wv_i[:, h * VW: h * VW + DH] = wv_g[:, h * DH:(h + 1) * DH]
            bv_i[0, h * VW: h * VW + DH] = bv_g[h * DH:(h + 1) * DH]
            bv_i[0, h * VW + DH] = 1.0
        mb = np.concatenate([
            b_qkv[0 * D:1 * D][sl].reshape(FT, 128).T,
            b_qkv[1 * D:2 * D][sl].reshape(FT, 128).T,
        ], axis=1)                                   # [128, 2*FT] f32
        in_maps.append({
            "xTp": xp[b],
            "wq": pack_k(W_qkv[:, 0 * D:1 * D][:, sl].astype(np.float16)),
            "wk": pack_k(W_qkv[:, 1 * D:2 * D][:, sl].astype(np.float16)),
            "wv": pack_k(wv_i),
            "mb": np.ascontiguousarray(mb),
            "bv": bv_i,
            "wo": np.ascontiguousarray(
                W_out[sl, :].astype(np.float16).reshape(FT, 128, D)
                .transpose(1, 0, 2).reshape(128, FT * D)
            ),
        })

    kw = {}
    if os.environ.get("KERNEL_TRACE") == "1":
        kw["trace"] = True
    res = run_bass_kernel_spmd(nc, in_maps, core_ids=list(range(NCORES)), **kw)
    LAST_RESULT = res

    out = np.empty((B, T, D), np.float32)
    for b in range(B):
        acc = res.results[GROUPS * b]["out"].astype(np.float32)
        for g in range(1, GROUPS):
            acc = acc + res.results[GROUPS * b + g]["out"]
        out[b] = acc + b_out
    return out

